# revision 33
# baseline (speedup 1.0000x reference)
"""AdaptiveAttention (B=2, S=2048, D=1024, H=16) on 8 TRN2 NeuronCores.

Sharding: query-parallel. Core c (c = 0..7) owns batch b = c//4 and query rows
[512*(c%4), 512*(c%4+1)). K projections are computed for the core's own 512
key rows, quantized to fp8e4m3, and AllGathered within each batch group of 4
cores as two pieces ({2,6} dims-tiles) so attention can start as soon as the
first piece lands; V stays bf16 and follows as three pieces ({2,3,3}). Each
core then computes all 16 heads of attention for its 512 queries against the
full 2048 keys, applies the per-(head, query) gate / softmax-denominator scale
to the context, and runs the full output projection for its rows. The host
concatenates the 8 disjoint [512, 1024] output blocks.

On-chip layout is "feature-major" (transposed): projections produce Q^T/K^T
directly so scores come out keys-on-partitions, which feeds exp (ScalarE, with
the 1/sqrt(dk) folded into the activation scale) and the P@V matmul without
any on-device transposes. Score matmuls are M=128/K=64 (one per head per
key-tile, two concurrent PE row-groups). The sigmoid gate is computed via the
exp table (sigmoid(g)/denom == 1/((1+exp(-g))*denom)) so the scalar engine
never swaps activation-table sets; the combined reciprocal uses the fast DVE
approximation and is broadcast across partitions on GpSimd (no DRAM bounce).
The output projection runs as 7-term chains (pairs 0-6) overlapping the last
pair's attention, with a short pair-7 finish; output DMA is spread across
four engine queues.
"""

import contextlib
import ctypes
import os
import sys
import types

import numpy as np
import ml_dtypes


def _install_ntff_hook_shim():
    """Provide antenv.axon_hooks (absent in this image) so
    run_bass_kernel_spmd(trace=True) can capture NTFF profiles."""
    if "antenv.axon_hooks" in sys.modules:
        return
    try:
        lib = ctypes.CDLL("/opt/axon/libaxon_pjrt.so")
    except OSError:
        return
    if not hasattr(lib, "axon_start_nrt_profile"):
        return
    lib.axon_start_nrt_profile.argtypes = [
        ctypes.POINTER(ctypes.c_int64),
        ctypes.c_size_t,
    ]
    lib.axon_start_nrt_profile.restype = ctypes.c_int64
    lib.axon_stop_nrt_profile.argtypes = [ctypes.c_char_p]
    lib.axon_stop_nrt_profile.restype = ctypes.c_int64

    @contextlib.contextmanager
    def _hook(output_dir, device_ids):
        import jax

        jax.devices()
        if device_ids:
            ids = (ctypes.c_int64 * len(device_ids))(*device_ids)
            rc = lib.axon_start_nrt_profile(ids, len(device_ids))
        else:
            rc = lib.axon_start_nrt_profile(None, 0)
        if rc != 0:
            raise RuntimeError(f"axon_start_nrt_profile rc={rc}")
        try:
            yield
        finally:
            n = lib.axon_stop_nrt_profile(str(output_dir).encode())
            if n < 0:
                raise RuntimeError(f"axon_stop_nrt_profile rc={n}")

    mod = types.ModuleType("antenv.axon_hooks")
    _state = {"hook": _hook}
    mod.get_axon_ntff_profile_hook = lambda: _state["hook"]
    mod.set_axon_ntff_profile_hook = lambda h: _state.__setitem__("hook", h)
    sys.modules["antenv.axon_hooks"] = mod
    try:
        import antenv

        antenv.axon_hooks = mod
    except ImportError:
        pass


_install_ntff_hook_shim()

import concourse.bass as bass  # noqa: E402
import concourse.mybir as mybir  # noqa: E402
import concourse.tile as tile  # noqa: E402
from concourse import bacc  # noqa: E402
from concourse.bass_utils import run_bass_kernel_spmd  # noqa: E402

# ---------------------------------------------------------------------------
# Problem constants (hardcoded; kernel.py must be self-contained)
# ---------------------------------------------------------------------------
B, S, D, H = 2, 2048, 1024, 16
DK = D // H                  # 64
N_CORES = 8
R = 4                        # ranks per batch group
SL = S // R                  # 512 local rows per core
P = 128
DT = D // P                  # 8 feature tiles
NKT = S // P                 # 16 key tiles
SCALE = DK ** -0.5

F32 = mybir.dt.float32
BF16 = mybir.dt.bfloat16
FP8 = mybir.dt.float8e4
KDT = mybir.dt.bfloat16 if os.environ.get("KBF16") == "1" else mybir.dt.float8e4
AF = mybir.ActivationFunctionType
ALU = mybir.AluOpType
BF16_NP = ml_dtypes.bfloat16

# AG pieces over dims-tiles: K pieces first (fp8), then V pieces (bf16).
K_PIECES = [(0, 8)]
V_PIECES = [(0, 2), (2, 6), (6, 8)]

_CACHE = {}
LAST_EXEC_TIME_NS = None


def _build():
    nc = bacc.Bacc("TRN2", target_bir_lowering=False, debug=False,
                   num_devices=N_CORES)

    # ---- I/O --------------------------------------------------------------
    xqT = nc.dram_tensor("xqT", [D, SL], BF16, kind="ExternalInput")
    xkT = nc.dram_tensor("xkT", [D, SL], BF16, kind="ExternalInput")
    xvT = nc.dram_tensor("xvT", [D, SL], BF16, kind="ExternalInput")
    # column-block packed W: [mt, p, kt, 128]
    wqp = nc.dram_tensor("wqp", [DT, P, DT, P], BF16, kind="ExternalInput")
    wkp = nc.dram_tensor("wkp", [DT, P, DT, P], BF16, kind="ExternalInput")
    # half packed W (rhs layout): [c2, p, kt, 512]
    wvp = nc.dram_tensor("wvp", [2, P, DT, 512], BF16, kind="ExternalInput")
    wo = nc.dram_tensor("wo", [D, D], BF16, kind="ExternalInput")
    wg = nc.dram_tensor("wg", [P, DT, H], BF16, kind="ExternalInput")
    bq = nc.dram_tensor("bq", [P, DT], F32, kind="ExternalInput")
    bk = nc.dram_tensor("bk", [P, DT], F32, kind="ExternalInput")
    bvb = nc.dram_tensor("bvb", [P, D], BF16, kind="ExternalInput")
    bob = nc.dram_tensor("bob", [P, D], BF16, kind="ExternalInput")
    nbg = nc.dram_tensor("nbg", [P, 1], F32, kind="ExternalInput")
    out = nc.dram_tensor("out", [SL, D], BF16, kind="ExternalOutput")
    DBG = os.environ.get("DUMPDBG") == "1"
    if DBG:
        dbg_den = nc.dram_tensor("dbg_den", [P, SL], F32,
                                 kind="ExternalOutput")
        dbg_gal = nc.dram_tensor("dbg_gal", [P, SL], F32,
                                 kind="ExternalOutput")
        dbg_ctx = nc.dram_tensor("dbg_ctx", [P, DT, SL], BF16,
                                 kind="ExternalOutput")
        dbg_e = nc.dram_tensor("dbg_e", [P, SL], F32, kind="ExternalOutput")
        dbg_kt = nc.dram_tensor("dbg_kt", [P, DT, R, SL], KDT,
                                kind="ExternalOutput")
        dbg_vw = nc.dram_tensor("dbg_vw", [P, R, DT, R, P], BF16,
                                kind="ExternalOutput")

    GROUPS = [[0, 1, 2, 3], [4, 5, 6, 7]]

    with tile.TileContext(nc) as tc, nc.allow_low_precision("fp8 K allgather"):
        with (
            tc.tile_pool(name="cst", bufs=1) as cst,
            tc.tile_pool(name="wpool", bufs=1) as wpool,
            tc.tile_pool(name="kvpool", bufs=1) as kvpool,
            tc.tile_pool(name="work", bufs=2) as work,
            tc.tile_pool(name="psA", bufs=1, space="PSUM") as psA,
            tc.tile_pool(name="psB", bufs=3, space="PSUM") as psB,
            tc.tile_pool(name="psC", bufs=1, space="PSUM") as psC,
            tc.tile_pool(name="dram", bufs=1, space="DRAM") as dram,
        ):
            # psB "sc" tiles are [P, 2, 512]; projection chains borrow one
            # 512-column slice of the same slots.
            def proj_psum():
                t = psB.tile([P, 2, 512], F32, tag="sc")
                return t[:, 0, :]

            # ---- constants / biases --------------------------------------
            bk_sb = cst.tile([P, DT], F32, name="bk_sb")
            nc.sync.dma_start(bk_sb[:], bk[:])
            bq_sb = cst.tile([P, DT], F32, name="bq_sb")
            nc.sync.dma_start(bq_sb[:], bq[:])
            nbg_sb = cst.tile([P, 1], F32, name="nbg_sb")
            nc.sync.dma_start(nbg_sb[:], nbg[:])
            ones_sb = cst.tile([P, 1], BF16, name="ones_sb")
            nc.vector.memset(ones_sb[:], 1.0)

            # ---- input loads, ordered for earliest K-proj start ----------
            xw_ctx = tc.tile_pool(name="xw", bufs=1)
            xw = xw_ctx.__enter__()
            wk_sb = xw.tile([P, DT, DT, P], BF16, name="wk_sb")   # [p,mt,kt,c]
            wq_sb = xw.tile([P, DT, DT, P], BF16, name="wq_sb")
            wv_sb = xw.tile([P, 2, DT, 512], BF16, name="wv_sb")  # [p,c2,kt,n]
            xk_sb = xw.tile([P, DT, SL], BF16, name="xk_sb")
            xv_sb = xw.tile([P, DT, SL], BF16, name="xv_sb")
            xq_sb = xw.tile([P, DT, SL], BF16, name="xq_sb")

            def _xsrc(d):
                return d.ap().rearrange("(t p) f -> p t f", p=P)

            # sync queue: xk then K weights (K-proj chain), then wv
            for c in range(4):
                sl = slice(2 * c, 2 * c + 2)
                nc.sync.dma_start(xk_sb[:, sl, :], _xsrc(xkT)[:, sl, :])
            for mt in range(DT):
                nc.sync.dma_start(wk_sb[:, mt, :, :], wkp.ap()[mt][:, :, :])
            nc.sync.dma_start(wv_sb[:, 0, :, :], wvp.ap()[0][:, :, :])
            nc.sync.dma_start(wv_sb[:, 1, :, :], wvp.ap()[1][:, :, :])
            # scalar queue: xq + Q weights, then xv, then O-proj weights
            for c in range(4):
                sl = slice(2 * c, 2 * c + 2)
                nc.scalar.dma_start(xv_sb[:, sl, :], _xsrc(xvT)[:, sl, :])
            for c in range(4):
                sl = slice(2 * c, 2 * c + 2)
                nc.scalar.dma_start(xq_sb[:, sl, :], _xsrc(xqT)[:, sl, :])
            for mt in range(DT):
                nc.scalar.dma_start(wq_sb[:, mt, :, :], wqp.ap()[mt][:, :, :])
            wg_sb = cst.tile([P, DT, H], BF16, name="wg_sb")
            nc.scalar.dma_start(wg_sb[:], wg[:])
            bv_sb = cst.tile([P, D], BF16, name="bv_sb")
            nc.scalar.dma_start(bv_sb[:], bvb[:])
            wo_sb = wpool.tile([P, DT, D], BF16, tag="wmat", name="wo_sb")
            wo_src = wo.ap().rearrange("(t p) f -> p t f", p=P)
            for c in range(4):
                sl = slice(2 * c, 2 * c + 2)
                nc.scalar.dma_start(wo_sb[:, sl, :], wo_src[:, sl, :])
            bo_sb = cst.tile([P, D], BF16, name="bo_sb")
            nc.scalar.dma_start(bo_sb[:], bob[:])

            # ---- local K^T (fp8) / V projections -------------------------
            ktloc = kvpool.tile([P, DT, SL], KDT, name="ktloc")
            vloc = kvpool.tile([P, DT, R, P], BF16, name="vloc")

            def k_proj(mt):
                pp = proj_psum()
                for kt in range(DT):
                    nc.tensor.matmul(pp[:], wk_sb[:, mt, kt, :],
                                     xk_sb[:, kt, :],
                                     start=(kt == 0), stop=(kt == DT - 1))
                nc.vector.tensor_scalar_add(ktloc[:, mt, :], pp[:],
                                            bk_sb[:, mt:mt + 1])

            def v_proj(kb, c2):
                pp = proj_psum()
                for kt in range(DT):
                    nc.tensor.matmul(
                        pp[:], xv_sb[:, kt, 128 * kb:128 * kb + 128],
                        wv_sb[:, c2, kt, :],
                        start=(kt == 0), stop=(kt == DT - 1))
                nc.vector.tensor_add(
                    vloc[:, 4 * c2:4 * c2 + 4, kb, :],
                    pp[:].rearrange("p (w d) -> p w d", w=4),
                    bv_sb[:, 512 * c2:512 * c2 + 512].rearrange(
                        "p (w d) -> p w d", w=4))

            # ---- AG buffers ----------------------------------------------
            kin = [dram.tile([P, (pe - ps) * SL], KDT, name=f"kin{i}")
                   for i, (ps, pe) in enumerate(K_PIECES)]
            kout = [dram.tile([R, P, (pe - ps) * SL], KDT, name=f"kout{i}")
                    for i, (ps, pe) in enumerate(K_PIECES)]
            vin = [dram.tile([P, (pe - ps) * R * P], BF16, name=f"vin{i}")
                   for i, (ps, pe) in enumerate(V_PIECES)]
            vout = [dram.tile([R, P, (pe - ps) * R * P], BF16,
                              name=f"vout{i}")
                    for i, (ps, pe) in enumerate(V_PIECES)]

            def issue_k_piece(i):
                ps_, pe_ = K_PIECES[i]
                n = pe_ - ps_
                nc.gpsimd.dma_start(
                    kin[i].rearrange("p (t k) -> p t k", t=n),
                    ktloc[:, ps_:pe_, :])
                nc.gpsimd.collective_compute(
                    "AllGather", ALU.bypass, replica_groups=GROUPS,
                    ins=[kin[i].opt()], outs=[kout[i].opt()])

            def issue_v_piece(i):
                ps_, pe_ = V_PIECES[i]
                n = pe_ - ps_
                nc.gpsimd.dma_start(
                    vin[i].rearrange("p (w a d) -> p w a d", w=n, a=R),
                    vloc[:, ps_:pe_, :, :])
                nc.gpsimd.collective_compute(
                    "AllGather", ALU.bypass, replica_groups=GROUPS,
                    ins=[vin[i].opt()], outs=[vout[i].opt()])

            # K proj -> K piece 0, V (c2=0) -> V piece 0, rest of K ->
            # K piece 1; V pieces 1,2 follow. CC queue order: K0 V0 K1 V1 V2
            qt_sb = cst.tile([P, DT, SL], BF16, name="qt_sb")

            def q_proj(mt):
                pp = proj_psum()
                for kt in range(DT):
                    nc.tensor.matmul(pp[:], wq_sb[:, mt, kt, :],
                                     xq_sb[:, kt, :],
                                     start=(kt == 0), stop=(kt == DT - 1))
                nc.vector.tensor_scalar_add(qt_sb[:, mt, :], pp[:],
                                            bq_sb[:, mt:mt + 1])

            for mt in range(DT):
                k_proj(mt)
            issue_k_piece(0)
            for kb in range(R):
                v_proj(kb, 0)
            issue_v_piece(0)
            for mt in range(DT):
                q_proj(mt)

            # e = exp(-(g + bg)); gate/denom = 1/((1+e)*denom) later.
            e_sb = cst.tile([P, SL], F32, name="e_sb")
            if DBG:
                nc.vector.memset(e_sb[:], 0)
            gp = proj_psum()
            for kt in range(DT):
                nc.tensor.matmul(gp[0:16, :], wg_sb[:, kt, :],
                                 xq_sb[:, kt, :],
                                 start=(kt == 0), stop=(kt == DT - 1))
            nc.scalar.activation(e_sb[0:16, :], gp[0:16, :], AF.Exp,
                                 bias=nbg_sb[0:16, 0:1], scale=-1.0)

            # ---- V projections second half + remaining V pieces ----------
            for kb in range(R):
                v_proj(kb, 1)
            issue_v_piece(1)
            issue_v_piece(2)

            xw_ctx.__exit__(None, None, None)
            kvr_ctx = tc.tile_pool(name="kvrecv", bufs=1)
            kvr = kvr_ctx.__enter__()
            ptp_ctx = tc.tile_pool(name="pt_pool", bufs=5)
            ptp = ptp_ctx.__enter__()

            # ---- gathered K/V receive tiles ------------------------------
            ktw = kvr.tile([P, DT, R, SL], KDT, name="ktw")
            vw = kvr.tile([P, R, DT, R, P], BF16, name="vw")
            recv = []
            for i, (ps_, pe_) in enumerate(K_PIECES):
                recv.append((K_PIECES[i][0], "k", i))
            for i, (ps_, pe_) in enumerate(V_PIECES):
                recv.append((V_PIECES[i][0], "v", i))
            # arrival order on the CC stream: K0 V0 V1 V2
            for _, kind, i in [(0, "k", 0), (0, "v", 0),
                               (2, "v", 1), (5, "v", 2)]:
                if kind == "k":
                    ps_, pe_ = K_PIECES[i]
                    n = pe_ - ps_
                    for r_ in range(R):
                        nc.sync.dma_start(
                            ktw[:, ps_:pe_, r_, :],
                            kout[i][r_].rearrange("p (t k) -> p t k", t=n))
                else:
                    ps_, pe_ = V_PIECES[i]
                    n = pe_ - ps_
                    for r_ in range(R):
                        nc.sync.dma_start(
                            vw[:, r_, ps_:pe_, :, :],
                            vout[i][r_].rearrange(
                                "p (w a d) -> p w a d", w=n, a=R))

            # ---- attention, 8 head-pairs pipelined -----------------------
            ctxT = cst.tile([P, DT, SL], BF16, name="ctxT")
            oacc = cst.tile([P, SL // P, 2, 512], BF16, name="oacc")
            denoms = cst.tile([P, SL], F32, name="denoms")
            gal = cst.tile([P, SL], F32, name="gal")
            galt = cst.tile([P, SL], F32, name="galt")
            if DBG:
                nc.vector.memset(denoms[:], 0)
                nc.vector.memset(gal[:], 0)

            def hpart(h):
                return h

            def emit_pv_mm(st, tg):
                # P@V matmuls for 2 keytiles
                pair = st["pair"]
                for par in (0, 1):
                    tau = 2 * tg + par
                    vt = vw[:, tau // R, pair, tau % R, :]
                    nc.tensor.matmul(
                        st["cp"][64:128, :], vt[:, 64:128],
                        st["ptB"][:, tau, :],
                        start=(tau == 0), stop=(tau == NKT - 1),
                        tile_position=(0, 64), skip_group_check=True)
                    nc.tensor.matmul(
                        st["cp"][0:64, :], vt[:, 0:64],
                        st["ptA"][:, tau, :],
                        start=(tau == 0), stop=(tau == NKT - 1),
                        tile_position=(0, 0), skip_group_check=True)

            def emit_sums(st, tg):
                # denominator (ones-row) matmuls for 2 keytiles
                for j, pt_t in ((0, st["ptA"]), (1, st["ptB"])):
                    for par in (0, 1):
                        colg = 2 * j + par
                        tau = 2 * tg + par
                        nc.tensor.matmul(
                            st["sums"][32 * colg:32 * colg + 1, :],
                            ones_sb[:, 0:1], pt_t[:, tau, :],
                            start=(tg == 0), stop=(tg == NKT // 2 - 1),
                            tile_position=(0, 32 * colg),
                            skip_group_check=True)

            def emit_pv(st, tg):
                emit_pv_mm(st, tg)
                emit_sums(st, tg)

            def emit_denom(st):
                # combine the 4 denominator rows and DMA them to the
                # head-aligned rows of the denoms tile
                pair = st["pair"]
                pA, pB = hpart(2 * pair), hpart(2 * pair + 1)
                sums_pp = st["sums"]
                dtmp = work.tile([P, 512], F32, tag="dtmp")
                nc.vector.tensor_copy(dtmp[0:1, :], sums_pp[32:33, :])
                nc.vector.tensor_copy(dtmp[64:65, :], sums_pp[96:97, :])
                nc.vector.tensor_add(dtmp[0:1, :], sums_pp[0:1, :],
                                     dtmp[0:1, :])
                nc.vector.tensor_add(dtmp[64:65, :], sums_pp[64:65, :],
                                     dtmp[64:65, :])
                nc.gpsimd.dma_start(denoms[pA:pA + 1, :], dtmp[0:1, :])
                nc.gpsimd.dma_start(denoms[pB:pB + 1, :], dtmp[64:65, :])

            def emit_copy(st):
                nc.vector.tensor_copy(ctxT[:, st["pair"], :], st["cp"][:, :])
                emit_denom(st)

            def emit_gal(st):
                # gal = 1/((1+e)*denom); broadcast each head's row across 64
                # partitions via DRAM-bounce DMAs; returns the bcast tile
                pair = st["pair"]
                pA = hpart(2 * pair)
                # DVE partition base must be 0 on HW for these ops:
                # recompute all rows so far each time (cost scales with free
                # dim only, so the redundancy is free)
                rows = slice(0, 2 * pair + 2)
                nc.vector.scalar_tensor_tensor(
                    galt[rows, :], e_sb[rows, :], 1.0, denoms[rows, :],
                    op0=ALU.add, op1=ALU.mult)
                nc.vector.reciprocal_approx_fast(gal[rows, :], galt[rows, :])
                sbc = work.tile([P, 512], F32, tag="sbc")
                srow_d = dram.tile([2, 512], F32, name=f"srow_d{pair}")
                nc.gpsimd.dma_start(srow_d[:, :], gal[pA:pA + 2, :])
                nc.gpsimd.dma_start(sbc[0:64, :],
                                    srow_d[0:1, :].to_broadcast([64, 512]))
                nc.gpsimd.dma_start(sbc[64:128, :],
                                    srow_d[1:2, :].to_broadcast([64, 512]))
                return sbc

            def emit_mul(st, sbc):
                nc.vector.tensor_mul(ctxT[:, st["pair"], :],
                                     ctxT[:, st["pair"], :], sbc[:, :])

            def emit_scale(st):
                emit_mul(st, emit_gal(st))

            def new_state(pair):
                return {
                    "pair": pair,
                    "ptA": ptp.tile([P, NKT, SL], BF16, tag="pt",
                                    name="ptA"),
                    "ptB": ptp.tile([P, NKT, SL], BF16, tag="pt",
                                    name="ptB"),
                    "cp": psA.tile([P, 512], F32, tag="pc", name="cp_ps"),
                    "sums": psC.tile([P, 512], F32, tag="sums",
                                     name="sums_ps"),
                }

            pend = None
            for pair in range(DT):
                st = new_state(pair)
                for tg in range(NKT // 2):
                    sA = psB.tile([P, 2, 512], F32, tag="sc")
                    sB = psB.tile([P, 2, 512], F32, tag="sc")
                    for j in (0, 1):
                        tau = 2 * tg + j
                        r_, kl = tau // R, tau % R
                        if os.environ.get("SC_M128") == "1":
                            ks = slice(128 * kl, 128 * kl + 128)
                            nc.tensor.matmul(
                                sB[:, j, :], ktw[64:128, pair, r_, ks],
                                qt_sb[64:128, pair, :],
                                start=True, stop=True, tile_position=(64, 0))
                            nc.tensor.matmul(
                                sA[:, j, :], ktw[0:64, pair, r_, ks],
                                qt_sb[0:64, pair, :],
                                start=True, stop=True, tile_position=(0, 0))
                        else:
                            klo = slice(128 * kl, 128 * kl + 64)
                            khi = slice(128 * kl + 64, 128 * kl + 128)
                            nc.tensor.matmul(
                                sB[0:64, j, :], ktw[64:128, pair, r_, klo],
                                qt_sb[64:128, pair, :],
                                start=True, stop=True,
                                tile_position=(64, 0))
                            nc.tensor.matmul(
                                sB[64:128, j, :], ktw[64:128, pair, r_, khi],
                                qt_sb[64:128, pair, :],
                                start=True, stop=True,
                                tile_position=(64, 64))
                            nc.tensor.matmul(
                                sA[0:64, j, :], ktw[0:64, pair, r_, klo],
                                qt_sb[0:64, pair, :],
                                start=True, stop=True,
                                tile_position=(0, 0))
                            nc.tensor.matmul(
                                sA[64:128, j, :], ktw[0:64, pair, r_, khi],
                                qt_sb[0:64, pair, :],
                                start=True, stop=True,
                                tile_position=(0, 64))
                    nc.scalar.activation(st["ptA"][:, 2 * tg:2 * tg + 2, :],
                                         sA[:, :, :], AF.Exp, scale=SCALE)
                    nc.scalar.activation(st["ptB"][:, 2 * tg:2 * tg + 2, :],
                                         sB[:, :, :], AF.Exp, scale=SCALE)
                    if pend is not None:
                        emit_pv(pend, tg)
                if pend is not None:
                    emit_copy(pend)
                    emit_scale(pend)
                pend = st

            # last pair's P@V trails (V piece 2); the pair 0-6 O-projection
            # chains interleave so the PE fills the exp wait
            def oproj_chain(qi, c2):
                po = proj_psum()
                for pr in range(7):
                    nc.tensor.matmul(
                        po[:], ctxT[:, pr, 128 * qi:128 * qi + 128],
                        wo_sb[:, pr, 512 * c2:512 * c2 + 512],
                        start=(pr == 0), stop=(pr == 6))
                nc.vector.tensor_add(
                    oacc[:, qi, c2, :], po[:],
                    bo_sb[:, 512 * c2:512 * c2 + 512])

            # last pair: sums first (need only pt), so the denominator /
            # gal / broadcast chain overlaps the P@V matmuls
            for tg in range(NKT // 2):
                emit_sums(pend, tg)
            emit_denom(pend)
            sbc7 = emit_gal(pend)
            for tg in range(NKT // 2):
                emit_pv_mm(pend, tg)
                oproj_chain(tg // 2, tg % 2)
            nc.vector.tensor_copy(ctxT[:, 7, :], pend["cp"][:, :])
            emit_mul(pend, sbc7)
            out_engs = [nc.sync, nc.scalar, nc.gpsimd]
            for qi in range(SL // P):
                for c2 in range(2):
                    po = proj_psum()
                    nc.tensor.matmul(
                        po[:], ctxT[:, 7, 128 * qi:128 * qi + 128],
                        wo_sb[:, 7, 512 * c2:512 * c2 + 512],
                        start=True, stop=True)
                    osb = work.tile([P, 512], BF16, tag="osb")
                    nc.vector.tensor_add(osb[:, :], po[:],
                                         oacc[:, qi, c2, :])
                    out_engs[(2 * qi + c2) % 3].dma_start(
                        out[128 * qi:128 * qi + 128,
                            512 * c2:512 * c2 + 512],
                        osb[:])
            if DBG:
                nc.sync.dma_start(dbg_den[:], denoms[:])
                nc.sync.dma_start(dbg_gal[:], gal[:])
                nc.sync.dma_start(dbg_ctx[:], ctxT[:])
                nc.sync.dma_start(dbg_e[:], e_sb[:])
                nc.sync.dma_start(dbg_kt[:], ktw[:])
                nc.sync.dma_start(dbg_vw[:], vw[:])
            ptp_ctx.__exit__(None, None, None)
            kvr_ctx.__exit__(None, None, None)

    nc.compile()
    return nc


def _prep_inputs(query, key_, value, Wq, bq, Wk, bk, Wv, bv, Wo, bo, Wg, bg):
    """Host-side sharding / layout prep. Returns in_maps for the 8 cores."""
    f32 = np.float32

    def bf(x):
        return np.ascontiguousarray(np.asarray(x, f32)).astype(BF16_NP)

    def pack_cols(w):
        # [D, D] -> [mt, p, kt, 128]: column block mt as [p, kt, c]
        w4 = np.asarray(w, f32).reshape(DT, P, DT, P)     # [kt, p, mt, c]
        return np.ascontiguousarray(
            w4.transpose(2, 1, 0, 3)).astype(BF16_NP)

    def pack_halves(w):
        # [D, D] -> [c2, p, kt, 512]
        w4 = np.asarray(w, f32).reshape(DT, P, 2, 512)    # [kt, p, c2, n]
        return np.ascontiguousarray(
            w4.transpose(2, 1, 0, 3)).astype(BF16_NP)

    wq_b, wk_b, wv_b = pack_cols(Wq), pack_cols(Wk), pack_halves(Wv)
    wo_b = bf(Wo)
    wg_b = np.ascontiguousarray(bf(Wg).reshape(DT, P, H).transpose(1, 0, 2))
    bq_pm = np.ascontiguousarray(np.asarray(bq, f32).reshape(DT, P).T)
    bk_pm = np.ascontiguousarray(np.asarray(bk, f32).reshape(DT, P).T)
    bv_b = np.ascontiguousarray(
        np.broadcast_to(np.asarray(bv, f32).astype(BF16_NP), (P, D)))
    bo_b = np.ascontiguousarray(
        np.broadcast_to(np.asarray(bo, f32).astype(BF16_NP), (P, D)))
    nbg_c = np.zeros((P, 1), f32)
    bg_f = np.asarray(bg, f32).reshape(H)
    nbg_c[0:16, 0] = -bg_f[0:16]

    qT = [np.asarray(query[b], f32).T for b in range(B)]
    kT = [np.asarray(key_[b], f32).T for b in range(B)]
    vT = [np.asarray(value[b], f32).T for b in range(B)]

    in_maps = []
    for c in range(N_CORES):
        b, r = c // R, c % R
        rows = slice(SL * r, SL * (r + 1))
        in_maps.append({
            "xqT": np.ascontiguousarray(qT[b][:, rows]).astype(BF16_NP),
            "xkT": np.ascontiguousarray(kT[b][:, rows]).astype(BF16_NP),
            "xvT": np.ascontiguousarray(vT[b][:, rows]).astype(BF16_NP),
            "wqp": wq_b, "wkp": wk_b, "wvp": wv_b, "wo": wo_b, "wg": wg_b,
            "bq": bq_pm, "bk": bk_pm, "bvb": bv_b, "bob": bo_b, "nbg": nbg_c,
        })
    return in_maps


def kernel(query, key_, value, Wq, bq, Wk, bk, Wv, bv, Wo, bo, Wg, bg):
    global LAST_EXEC_TIME_NS
    if "nc" not in _CACHE:
        _CACHE["nc"] = _build()
    nc = _CACHE["nc"]

    in_maps = _prep_inputs(query, key_, value, Wq, bq, Wk, bk, Wv, bv,
                           Wo, bo, Wg, bg)
    trace = bool(os.environ.get("BASS_TRACE"))
    res = run_bass_kernel_spmd(nc, in_maps, core_ids=list(range(N_CORES)),
                               trace=trace)
    LAST_EXEC_TIME_NS = res.exec_time_ns

    out = np.empty((B, S, D), np.float32)
    for c in range(N_CORES):
        b, r = c // R, c % R
        out[b, SL * r:SL * (r + 1), :] = np.asarray(
            res.results[c]["out"], dtype=np.float32)
    return out
